# revision 1
# baseline (speedup 1.0000x reference)
"""CalderaLinear Trainium2 kernel.

Computes out = x @ dequant(q).T + (x @ dequant(r).T) @ dequant(l).T + bias
with groupwise (group=128) dequantization, distributed over 8 NeuronCores
by sharding tokens (batch*seq) 8 ways and replicating the weights.

Device does: dequant (scale multiply), both matmuls, low-rank path, bias add.
Host does: sharding + lossless int->bf16 casts of the quantized values
(0..15 are exact in bf16) + fp32->bf16 cast of x (the matmul precision
choice), and the final concat of per-core output shards.
"""

import os
import sys

import numpy as np
import ml_dtypes

for _p in ("/opt/trn_rl_repo",):
    if _p not in sys.path and os.path.isdir(_p):
        sys.path.insert(0, _p)

import concourse.bass as bass
import concourse.mybir as mybir
import concourse.tile as tile
from concourse import bacc
from concourse.bass_utils import run_bass_kernel_spmd

BF16 = mybir.dt.bfloat16
F32 = mybir.dt.float32

P = 128  # partitions / group size
N_CORES = 8

# Full problem shape (hardcoded per contest contract).
B, S, D_IN, D_OUT, RANK = 4, 2048, 4096, 4096, 256
N_TOK = B * S  # 8192
T_SH = N_TOK // N_CORES  # 1024 tokens per core


def caldera_tile_kernel(tc, out, x, qv, qs, lv, ls, rv, rs, bias_, obw=512):
    """One core's program. Shapes:
    x   [T, K]  bf16   (token shard, natural layout)
    qv  [O, K]  bf16   quantized values; qs [O, K/128] f32 scales
    lv  [O, R]  bf16 ; ls [O, R/128] f32
    rv  [R, K]  bf16 ; rs [R, K/128] f32
    bias_ [1, O] f32
    out [T, O]  f32
    """
    nc = tc.nc
    T, K = x.shape
    O = qv.shape[0]
    R = lv.shape[1]
    KC = K // P   # contraction chunks (== scale groups along K)
    RC = R // P   # rank chunks (== scale groups along R)
    TS = T // P   # token subtiles (psum partition dim)
    OBW = obw     # output-feature block width (moving free dim, psum N)
    NOB = O // OBW
    OBB = OBW // P  # 128-row blocks per output block
    TH = min(512, T)  # xr moving width
    NTH = T // TH
    XSPLIT = 4    # x_T loaded in XSPLIT transposes for earlier PE start
    rings = (nc.sync, nc.sync)  # single HWDGE ring (dual-ring corrupted on HW)

    with tc.tile_pool(name="const", bufs=1) as constp, \
         tc.tile_pool(name="stage", bufs=2) as stagep, \
         tc.tile_pool(name="qtp", bufs=2) as qtp, \
         tc.tile_pool(name="outp", bufs=6) as outp, \
         tc.tile_pool(name="psp", bufs=6, space="PSUM") as psp, \
         tc.tile_pool(name="psxr", bufs=2, space="PSUM") as psxrp:

        # ---- resident tensors ----
        x_T = constp.tile([P, KC, T], BF16)    # x.T chunks: x_T[p,g,t] = x[t, g*P+p]
        r_T = constp.tile([P, KC, R], BF16)    # r.T chunks: r_T[p,g,r] = r_deq[r, g*P+p]
        l_T = constp.tile([P, RC, O], BF16)    # l.T chunks: l_T[p,c,o] = l_deq[o, c*P+p]
        xr_T = constp.tile([P, RC, T], BF16)   # xr.T chunks: xr_T[p,c,t] = xr[t, c*P+p]
        ones = constp.tile([1, P], BF16)
        nc.vector.memset(ones[:], 1.0)
        bias_bf = constp.tile([1, O], BF16)
        # SWDGE dma casts f32 -> bf16 in flight
        nc.gpsimd.dma_start(out=bias_bf[:], in_=bias_[:])

        # ---- r: load + in-place dequant + one 3D transpose per block ----
        for rb in range(RC):
            rnat = stagep.tile([P, K], BF16, tag="nat")
            rsc = stagep.tile([P, KC], F32, tag="sc")
            nc.sync.dma_start(out=rnat[:], in_=rv[rb * P:(rb + 1) * P, :])
            nc.sync.dma_start(out=rsc[:], in_=rs[rb * P:(rb + 1) * P, :])
            nc.vector.tensor_tensor(
                out=rnat[:].rearrange("p (g j) -> p g j", g=KC),
                in0=rnat[:].rearrange("p (g j) -> p g j", g=KC),
                in1=rsc[:, :, None].broadcast_to([P, KC, P]),
                op=mybir.AluOpType.mult,
            )
            nc.sync.dma_start(
                out=r_T[:, :, rb * P:(rb + 1) * P], in_=rnat[:],
                transpose=True,
            )

        # ---- l: load + dequant + one 3D transpose per block ----
        for lb in range(O // P):
            lnat = stagep.tile([P, R], BF16, tag="lnat")
            lsc = stagep.tile([P, RC], F32, tag="lsc")
            nc.sync.dma_start(out=lnat[:], in_=lv[lb * P:(lb + 1) * P, :])
            nc.sync.dma_start(out=lsc[:], in_=ls[lb * P:(lb + 1) * P, :])
            ldeq = stagep.tile([P, R], BF16, tag="ldeq")
            nc.vector.tensor_tensor(
                out=ldeq[:].rearrange("p (c j) -> p c j", c=RC),
                in0=lnat[:].rearrange("p (c j) -> p c j", c=RC),
                in1=lsc[:, :, None].broadcast_to([P, RC, P]),
                op=mybir.AluOpType.mult,
            )
            nc.sync.dma_start(
                out=l_T[:, :, lb * P:(lb + 1) * P], in_=ldeq[:],
                transpose=True,
            )

        # ---- x: transposed loads straight from DRAM (split for early start) --
        kstep = KC // XSPLIT
        for xs in range(XSPLIT):
            nc.sync.dma_start(
                out=x_T[:, xs * kstep:(xs + 1) * kstep, :],
                in_=x[:, xs * kstep * P:(xs + 1) * kstep * P],
                transpose=True,
            )

        # ---- xr.T = (x @ r_deq.T).T, computed as r_chunk @ x.T ----
        for rb in range(RC):
            for th in range(NTH):
                pxr = psxrp.tile([P, TH], F32, tag="psxr")
                for g in range(KC):
                    nc.tensor.matmul(
                        pxr[:],
                        lhsT=r_T[:, g, rb * P:(rb + 1) * P],
                        rhs=x_T[:, g, th * TH:(th + 1) * TH],
                        start=(g == 0),
                        stop=(g == KC - 1),
                    )
                nc.scalar.copy(xr_T[:, rb, th * TH:(th + 1) * TH], pxr[:])

        # ---- main: stream q blocks, accumulate in PSUM ----
        for ob in range(NOB):
            q_T = qtp.tile([P, KC, OBW], BF16, tag="qT")
            for c in range(OBB):
                b = ob * OBB + c
                ring = rings[b % 2]
                qnat = stagep.tile([P, K], BF16, tag="nat")
                qsc = stagep.tile([P, KC], F32, tag="sc")
                ring.dma_start(out=qnat[:], in_=qv[b * P:(b + 1) * P, :])
                ring.dma_start(out=qsc[:], in_=qs[b * P:(b + 1) * P, :])
                nc.vector.tensor_tensor(
                    out=qnat[:].rearrange("p (g j) -> p g j", g=KC),
                    in0=qnat[:].rearrange("p (g j) -> p g j", g=KC),
                    in1=qsc[:, :, None].broadcast_to([P, KC, P]),
                    op=mybir.AluOpType.mult,
                )
                ring.dma_start(
                    out=q_T[:, :, c * P:(c + 1) * P], in_=qnat[:],
                    transpose=True,
                )
            for t in range(TS):
                ps = psp.tile([P, OBW], F32)
                for g in range(KC):
                    nc.tensor.matmul(
                        ps[:],
                        lhsT=x_T[:, g, t * P:(t + 1) * P],
                        rhs=q_T[:, g, :],
                        start=(g == 0),
                        stop=False,
                    )
                for c in range(RC):
                    nc.tensor.matmul(
                        ps[:],
                        lhsT=xr_T[:, c, t * P:(t + 1) * P],
                        rhs=l_T[:, c, ob * OBW:(ob + 1) * OBW],
                        start=False,
                        stop=False,
                    )
                nc.tensor.matmul(
                    ps[:],
                    lhsT=ones[:],
                    rhs=bias_bf[:, ob * OBW:(ob + 1) * OBW],
                    start=False,
                    stop=True,
                )
                osb = outp.tile([P, OBW], F32)
                nc.scalar.copy(osb[:], ps[:])
                rings[(ob + t) % 2].dma_start(
                    out=out[t * P:(t + 1) * P, ob * OBW:(ob + 1) * OBW],
                    in_=osb[:],
                )


def build_nc(T=T_SH, O=D_OUT, K=D_IN, R=RANK, obw=512):
    nc = bacc.Bacc("TRN2", target_bir_lowering=False, debug=False)
    x = nc.dram_tensor("x_sh", [T, K], BF16, kind="ExternalInput").ap()
    qv = nc.dram_tensor("qv", [O, K], BF16, kind="ExternalInput").ap()
    qs = nc.dram_tensor("qs", [O, K // P], F32, kind="ExternalInput").ap()
    lv = nc.dram_tensor("lv", [O, R], BF16, kind="ExternalInput").ap()
    ls = nc.dram_tensor("ls", [O, R // P], F32, kind="ExternalInput").ap()
    rv = nc.dram_tensor("rv", [R, K], BF16, kind="ExternalInput").ap()
    rs = nc.dram_tensor("rs", [R, K // P], F32, kind="ExternalInput").ap()
    bias_ = nc.dram_tensor("bias", [1, O], F32, kind="ExternalInput").ap()
    out = nc.dram_tensor("out", [T, O], F32, kind="ExternalOutput").ap()
    with tile.TileContext(nc) as tc:
        caldera_tile_kernel(tc, out, x, qv, qs, lv, ls, rv, rs, bias_,
                            obw=obw)
    nc.compile()
    return nc


def make_in_maps(x, q_values, q_scales, l_values, l_scales, r_values, r_scales,
                 bias):
    bf16 = ml_dtypes.bfloat16
    xf = np.ascontiguousarray(
        np.asarray(x, dtype=np.float32).reshape(N_TOK, D_IN)
    ).astype(bf16)
    qv = np.asarray(q_values).astype(bf16)  # ints 0..15: exact
    lv = np.asarray(l_values).astype(bf16)
    rv = np.asarray(r_values).astype(bf16)
    qs = np.ascontiguousarray(np.asarray(q_scales, dtype=np.float32))
    ls = np.ascontiguousarray(np.asarray(l_scales, dtype=np.float32))
    rs = np.ascontiguousarray(np.asarray(r_scales, dtype=np.float32))
    b = np.ascontiguousarray(
        np.asarray(bias, dtype=np.float32).reshape(1, D_OUT)
    )
    in_maps = []
    for i in range(N_CORES):
        in_maps.append({
            "x_sh": np.ascontiguousarray(xf[i * T_SH:(i + 1) * T_SH]),
            "qv": qv, "qs": qs, "lv": lv, "ls": ls,
            "rv": rv, "rs": rs, "bias": b,
        })
    return in_maps


_NC_CACHE = {}


def _get_nc():
    if "nc" not in _NC_CACHE:
        _NC_CACHE["nc"] = build_nc()
    return _NC_CACHE["nc"]


def run(inputs, trace=False, tmpdir=None):
    nc = _get_nc()
    in_maps = make_in_maps(**inputs)
    res = run_bass_kernel_spmd(
        nc, in_maps, list(range(N_CORES)), trace=trace, tmpdir=tmpdir
    )
    shards = [np.asarray(res.results[i]["out"]) for i in range(N_CORES)]
    full = np.concatenate(shards, axis=0).reshape(B, S, D_OUT)
    return full.astype(np.float32), res


def kernel(**inputs) -> np.ndarray:
    out, _ = run(inputs, trace=False)
    return out



# revision 7
# speedup vs baseline: 1.4015x; 1.4015x over previous
"""CalderaLinear Trainium2 kernel, v2 (fp8 DoubleRow q-path).

Computes out = x @ dequant(q).T + (x @ dequant(r).T) @ dequant(l).T + bias
with groupwise (group=128) dequantization, distributed over 8 NeuronCores
by sharding tokens (batch*seq) 8 ways and replicating the weights.

Device work: dequant (PE scale broadcast + DVE multiply), the big q-path
GEMM in fp8 DoubleRow mode (2 contraction groups per instruction), the
low-rank path in bf16, bias add fused into the PSUM drain.

Host work (layout/cast only): shard tokens, transpose weights/activations
to contraction-major layouts, lossless int->fp8/bf16 casts of quantized
values (0..15 exact in e4m3 and bf16), fp32->bf16/fp8 cast of x, scale
transposes, final concat of per-core output shards.
"""

import os
import sys

import numpy as np
import ml_dtypes

for _p in ("/opt/trn_rl_repo",):
    if _p not in sys.path and os.path.isdir(_p):
        sys.path.insert(0, _p)

import concourse.bass as bass
import concourse.mybir as mybir
import concourse.tile as tile
from concourse import bacc
from concourse.bass_utils import run_bass_kernel_spmd

BF16 = mybir.dt.bfloat16
F32 = mybir.dt.float32
FP8 = mybir.dt.float8e4
DR = mybir.MatmulPerfMode.DoubleRow

P = 128  # partitions / quant group size
N_CORES = 8

# Full problem shape (hardcoded per contest contract).
B, S, D_IN, D_OUT, RANK = 4, 2048, 4096, 4096, 256
N_TOK = B * S  # 8192
T_SH = N_TOK // N_CORES  # 1024 tokens per core


def caldera_v2(tc, out, xT8, xTb, qT8, qsS, rT, rsF, lT, lsF, bias_, obw=512):
    """One core's program. DRAM layouts (K=in, O=out, R=rank, T=tokens):
    xT8  [P, K/P, T]        fp8   x.T chunks: xT8[p,g,t] = x[t, g*P+p]
    xTb  [P, K/P, T]        bf16  same values in bf16 (low-rank path input)
    qT8  [NOB, P, K/P, OBW] fp8   q.T slab-major: [ob,p,g,oo] = q[ob*OBW+oo, g*P+p]
    qsS  [NOB, 1, K/P, OBW] bf16  q_scales slab-major: [ob,0,g,oo] = qs[ob*OBW+oo, g]
    rT   [P, K/P, R]        bf16  r.T chunks
    rsF  [1, K/P, R]        bf16  r_scales.T (flat, partition 0)
    lT   [P, R/P, O]        bf16  l.T chunks
    lsF  [1, R/P, O]        bf16  l_scales.T (flat, partition 0)
    bias_ [1, O]            bf16
    out  [T, O]             f32
    """
    nc = tc.nc
    T = xT8.shape[2]
    KC = xT8.shape[1]          # contraction groups (32)
    RC = lT.shape[1]           # rank groups (2)
    R = rT.shape[2]
    O = lT.shape[2]
    OBW = obw
    NOB = O // OBW             # output slabs (8)
    TS = T // P                # token tiles (8)
    NGP = KC // 2              # DoubleRow group pairs (16)

    with tc.tile_pool(name="const", bufs=1) as constp, \
         tc.tile_pool(name="qtp", bufs=2) as qtp, \
         tc.tile_pool(name="outp", bufs=4) as outp, \
         tc.tile_pool(name="psm", bufs=4, space="PSUM") as psm, \
         tc.tile_pool(name="pssc", bufs=2, space="PSUM") as pssc:

        # ---- resident tensors ----
        x8 = constp.tile([P, KC, T], FP8)
        rt = constp.tile([P, KC, R], BF16)
        lt = constp.tile([P, RC, O], BF16)
        xrT = constp.tile([P, RC, T], BF16)
        bias_bc = constp.tile([P, O], BF16)
        ones = constp.tile([1, P], BF16)
        nc.vector.memset(ones[:], 1.0)

        # ---- prologue (scoped pool; space reclaimed for main loop) ----
        with tc.tile_pool(name="pro", bufs=1) as prop:
            rs = prop.tile([1, KC, R], BF16)
            ls = prop.tile([1, RC, O], BF16)
            bb = prop.tile([1, O], BF16)

            # loads (small first, then r/l, then x-chunks for xr)
            nc.sync.dma_start(out=rs[:], in_=rsF[:])
            nc.sync.dma_start(out=ls[:], in_=lsF[:])
            nc.sync.dma_start(out=bb[:], in_=bias_[:])
            nc.sync.dma_start(out=rt[:], in_=rT[:])
            nc.sync.dma_start(out=lt[:], in_=lT[:])

            # bias broadcast: psum outer product, copy to sbuf
            for j in range(O // OBW):
                pb = pssc.tile([P, OBW], F32, tag="sc")
                nc.tensor.matmul(pb[:], lhsT=ones[:],
                                 rhs=bb[0:1, j * OBW:(j + 1) * OBW],
                                 start=True, stop=True)
                nc.vector.tensor_scalar_mul(bias_bc[:, j * OBW:(j + 1) * OBW],
                                            pb[:], 1.0)

            # r dequant: scale bcast via PE, DVE multiply in place
            for g in range(KC):
                psc = pssc.tile([P, R], F32, tag="sc")
                nc.tensor.matmul(psc[:], lhsT=ones[:],
                                 rhs=rs[0:1, g, :], start=True, stop=True)
                nc.vector.tensor_tensor(out=rt[:, g, :], in0=rt[:, g, :],
                                        in1=psc[:], op=mybir.AluOpType.mult)

            # l dequant
            for cg in range(RC):
                for j in range(O // OBW):
                    psc = pssc.tile([P, OBW], F32, tag="sc")
                    nc.tensor.matmul(psc[:], lhsT=ones[:],
                                     rhs=ls[0:1, cg, j * OBW:(j + 1) * OBW],
                                     start=True, stop=True)
                    nc.vector.tensor_tensor(
                        out=lt[:, cg, j * OBW:(j + 1) * OBW],
                        in0=lt[:, cg, j * OBW:(j + 1) * OBW],
                        in1=psc[:], op=mybir.AluOpType.mult)

            # xr.T = (x @ r_deq.T).T via r_chunk @ x.T, x streamed bf16
            # psum tiles [rh*P : , th*TH] held across the g accumulation.
            TH = min(512, T)
            NTH = T // TH
            NRH = R // P
            GBLK = min(8, KC)
            pxr = [[psm.tile([P, TH], F32, tag="mm", name=f"pxr_{rh}_{th}")
                    for th in range(NTH)]
                   for rh in range(NRH)]
            for gb in range(KC // GBLK):
                xc = qtp.tile([P, GBLK, T], BF16, tag="xbf")
                nc.sync.dma_start(out=xc[:],
                                  in_=xTb[:, gb * GBLK:(gb + 1) * GBLK, :])
                for gg in range(GBLK):
                    g = gb * GBLK + gg
                    for rh in range(NRH):
                        for th in range(NTH):
                            nc.tensor.matmul(
                                pxr[rh][th][:],
                                lhsT=rt[:, g, rh * P:(rh + 1) * P],
                                rhs=xc[:, gg, th * TH:(th + 1) * TH],
                                start=(g == 0), stop=(g == KC - 1),
                            )
            for rh in range(NRH):
                for th in range(NTH):
                    nc.scalar.copy(xrT[:, rh, th * TH:(th + 1) * TH],
                                   pxr[rh][th][:])

            # x.T fp8 (needed from main loop on)
            nc.sync.dma_start(out=x8[:], in_=xT8[:])

        # ---- main loop over output slabs ----
        with tc.tile_pool(name="qsp", bufs=1) as qsp:
            for ob in range(NOB):
                qsl = qtp.tile([P, KC, OBW], FP8, tag="q")
                nc.sync.dma_start(out=qsl[:], in_=qT8[ob])
                qss = qsp.tile([1, KC, OBW], BF16, tag="qs")
                nc.sync.dma_start(out=qss[:], in_=qsS[ob])
                # dequant: PE scale bcast -> psum, DVE multiply (fp8 in place)
                for g in range(KC):
                    psc = pssc.tile([P, OBW], F32, tag="sc")
                    nc.tensor.matmul(psc[:], lhsT=ones[:],
                                     rhs=qss[0:1, g, :], start=True, stop=True)
                    nc.vector.tensor_tensor(out=qsl[:, g, :], in0=qsl[:, g, :],
                                            in1=psc[:], op=mybir.AluOpType.mult)
                # matmuls
                for t in range(TS):
                    ps = psm.tile([P, OBW], F32, tag="mm")
                    for gp in range(NGP):
                        nc.tensor.matmul(
                            ps[:],
                            lhsT=x8[:, 2 * gp:2 * gp + 2, t * P:(t + 1) * P],
                            rhs=qsl[:, 2 * gp:2 * gp + 2, :],
                            start=(gp == 0), stop=False, perf_mode=DR,
                        )
                    for c in range(RC):
                        nc.tensor.matmul(
                            ps[:],
                            lhsT=xrT[:, c, t * P:(t + 1) * P],
                            rhs=lt[:, c, ob * OBW:(ob + 1) * OBW],
                            start=False, stop=(c == RC - 1),
                        )
                    osb = outp.tile([P, OBW], F32)
                    nc.vector.tensor_tensor(
                        out=osb[:], in0=ps[:],
                        in1=bias_bc[:, ob * OBW:(ob + 1) * OBW],
                        op=mybir.AluOpType.add)
                    nc.sync.dma_start(
                        out=out[t * P:(t + 1) * P, ob * OBW:(ob + 1) * OBW],
                        in_=osb[:],
                    )


def build_nc(T=T_SH, O=D_OUT, K=D_IN, R=RANK, obw=512):
    nc = bacc.Bacc("TRN2", target_bir_lowering=False, debug=False)
    KC = K // P
    RC = R // P
    NOB = O // obw
    xT8 = nc.dram_tensor("xT8", [P, KC, T], FP8, kind="ExternalInput").ap()
    xTb = nc.dram_tensor("xTb", [P, KC, T], BF16, kind="ExternalInput").ap()
    qT8 = nc.dram_tensor("qT8", [NOB, P, KC, obw], FP8, kind="ExternalInput").ap()
    qsS = nc.dram_tensor("qsS", [NOB, 1, KC, obw], BF16, kind="ExternalInput").ap()
    rT = nc.dram_tensor("rT", [P, KC, R], BF16, kind="ExternalInput").ap()
    rsF = nc.dram_tensor("rsF", [1, KC, R], BF16, kind="ExternalInput").ap()
    lT = nc.dram_tensor("lT", [P, RC, O], BF16, kind="ExternalInput").ap()
    lsF = nc.dram_tensor("lsF", [1, RC, O], BF16, kind="ExternalInput").ap()
    bias_ = nc.dram_tensor("bias", [1, O], BF16, kind="ExternalInput").ap()
    out = nc.dram_tensor("out", [T, O], F32, kind="ExternalOutput").ap()
    with tile.TileContext(nc) as tc:
        caldera_v2(tc, out, xT8, xTb, qT8, qsS, rT, rsF, lT, lsF, bias_,
                   obw=obw)
    nc.compile()
    return nc


def _chunked_T(a, part=P):
    """[N, K] -> [P, K//P, N] with out[p, g, n] = a[n, g*P+p]."""
    n, k = a.shape
    return np.ascontiguousarray(a.T.reshape(k // part, part, n).transpose(1, 0, 2))


def make_in_maps(x, q_values, q_scales, l_values, l_scales, r_values, r_scales,
                 bias, obw=512):
    bf16 = ml_dtypes.bfloat16
    f8 = ml_dtypes.float8_e4m3
    KC = D_IN // P
    RC = RANK // P
    NOB = D_OUT // obw

    xf = np.asarray(x, dtype=np.float32).reshape(N_TOK, D_IN)
    qv = np.asarray(q_values)
    # q.T slab-major fp8: [NOB, P, KC, OBW]
    qT = np.ascontiguousarray(
        qv.T.reshape(KC, P, NOB, obw).transpose(2, 1, 0, 3)).astype(f8)
    # q scales slab-major on partition 0: [NOB, 1, KC, OBW]
    qsS = np.ascontiguousarray(
        np.asarray(q_scales, dtype=np.float32).T  # [KC, O]
        .reshape(KC, NOB, obw).transpose(1, 0, 2)[:, None]).astype(bf16)
    rT = _chunked_T(np.asarray(r_values).astype(np.float32)).astype(bf16)
    rsF = np.ascontiguousarray(
        np.asarray(r_scales, dtype=np.float32).T[None]).astype(bf16)
    lT = _chunked_T(np.asarray(l_values).astype(np.float32)).astype(bf16)
    lsF = np.ascontiguousarray(
        np.asarray(l_scales, dtype=np.float32).T[None]).astype(bf16)
    b = np.asarray(bias, dtype=np.float32).reshape(1, D_OUT).astype(bf16)

    in_maps = []
    for i in range(N_CORES):
        xs = xf[i * T_SH:(i + 1) * T_SH]
        xT = _chunked_T(xs)
        in_maps.append({
            "xT8": xT.astype(f8),
            "xTb": xT.astype(bf16),
            "qT8": qT, "qsS": qsS,
            "rT": rT, "rsF": rsF, "lT": lT, "lsF": lsF,
            "bias": b,
        })
    return in_maps


_NC_CACHE = {}


def _get_nc():
    if "nc" not in _NC_CACHE:
        _NC_CACHE["nc"] = build_nc()
    return _NC_CACHE["nc"]


def run(inputs, trace=False, tmpdir=None):
    nc = _get_nc()
    in_maps = make_in_maps(**inputs)
    res = run_bass_kernel_spmd(
        nc, in_maps, list(range(N_CORES)), trace=trace, tmpdir=tmpdir
    )
    shards = [np.asarray(res.results[i]["out"]) for i in range(N_CORES)]
    full = np.concatenate(shards, axis=0).reshape(B, S, D_OUT)
    return full.astype(np.float32), res


def kernel(**inputs) -> np.ndarray:
    out, _ = run(inputs, trace=False)
    return out


# revision 12
# speedup vs baseline: 1.8565x; 1.3247x over previous
"""CalderaLinear Trainium2 kernel, v4 (fp8 DoubleRow q-path + lr path).

Computes out = x @ dequant(q).T + (x @ dequant(r).T) @ dequant(l).T + bias
with groupwise (group=128) dequantization, distributed over 8 NeuronCores
by sharding tokens (batch*seq) 8 ways and replicating the weights.

Device work: dequant (scale replication via stride-0 broadcast DMA + DVE
multiply), the q-path GEMM and the low-rank path in bf16 (quantization
noise there rides the coherent mean components of r/l and is heavily
amplified, so it stays 16-bit), bias add fused into the PSUM drain.

Host work (layout/cast only): shard tokens, transpose weights/activations
to contraction-major layouts, lossless int->fp8/bf16 casts of quantized
values (0..15 exact in e4m3 and bf16), fp32->fp8/bf16 casts of x, scale
transposes/casts, final concat of per-core output shards.
"""

import os
import sys

import numpy as np
import ml_dtypes

for _p in ("/opt/trn_rl_repo",):
    if _p not in sys.path and os.path.isdir(_p):
        sys.path.insert(0, _p)

import concourse.bass as bass
import concourse.mybir as mybir
import concourse.tile as tile
from concourse import bacc
from concourse.bass_utils import run_bass_kernel_spmd

BF16 = mybir.dt.bfloat16
F32 = mybir.dt.float32
FP8 = mybir.dt.float8e4
DR = mybir.MatmulPerfMode.DoubleRow

P = 128  # partitions / quant group size
N_CORES = 8

# Full problem shape (hardcoded per contest contract).
B, S, D_IN, D_OUT, RANK = 4, 2048, 4096, 4096, 256
N_TOK = B * S  # 8192
T_SH = N_TOK // N_CORES  # 1024 tokens per core


def caldera_v4(tc, out, xT8, xTb, qT8, qsS, rT, rsF, lT, lsF, bias_, obw=512):
    """One core's program. DRAM layouts (K=in, O=out, R=rank, T=tokens):
    xT8  [P, K/P, T]        fp8   x.T chunks: xT8[p,g,t] = x[t, g*P+p]
    xTb  [P, K/P, T]        bf16  same values in bf16 (low-rank path input)
    qT8  [NOB, P, K/P, OBW] fp8   q.T slab-major: [ob,p,g,oo] = q[ob*OBW+oo, g*P+p]
    qsS  [NOB, 1, K/P, OBW] fp8   q_scales slab-major: [ob,0,g,oo] = qs[ob*OBW+oo, g]
    rT   [P, K/P, R]        bf16  r.T chunks
    rsF  [1, K/P, R]        bf16  r_scales.T (flat, partition 0)
    lT   [P, R/P, O]        bf16  l.T chunks
    lsF  [1, R/P, O]        bf16  l_scales.T (flat, partition 0)
    bias_ [1, O]            bf16
    out  [T, O]             f32
    """
    nc = tc.nc
    T = xT8.shape[2]
    KC = xT8.shape[1]          # contraction groups (32)
    RC = lT.shape[1]           # rank groups (2)
    R = rT.shape[2]
    O = lT.shape[2]
    OBW = obw
    NOB = O // OBW             # output slabs (8)
    TS = T // P                # token tiles (8)
    NGP = KC // 2              # DoubleRow group pairs (16)

    with tc.tile_pool(name="const", bufs=1) as constp, \
         tc.tile_pool(name="qtp", bufs=2) as qtp, \
         tc.tile_pool(name="outp", bufs=4) as outp, \
         tc.tile_pool(name="psm", bufs=8, space="PSUM") as psm:

        # ---- resident tensors ----
        x8 = constp.tile([P, KC, T], FP8)
        rt = constp.tile([P, KC, R], BF16)
        lt = constp.tile([P, RC, O], BF16)
        xrT = constp.tile([P, RC, T], BF16)
        bias_bc = constp.tile([P, O], BF16)

        # bias broadcast straight from DRAM (stride-0 read)
        nc.sync.dma_start(out=bias_bc[:],
                          in_=bias_[0:1, None, :].broadcast_to([1, P, O]))

        # ---- prologue (scoped pool; space reclaimed for main loop) ----
        with tc.tile_pool(name="pro", bufs=1) as prop:
            # r scales replicated across partitions (DMA broadcast from DRAM)
            reps_r = prop.tile([P, KC, R], BF16)
            nc.sync.dma_start(
                out=reps_r[:],
                in_=rsF[0:1, None, :, :].broadcast_to([1, P, KC, R]))
            nc.sync.dma_start(out=rt[:], in_=rT[:])
            nc.sync.dma_start(out=x8[:], in_=xT8[:])
            nc.sync.dma_start(out=lt[:], in_=lT[:])

            # r dequant in one DVE op
            nc.vector.tensor_tensor(out=rt[:], in0=rt[:], in1=reps_r[:],
                                    op=mybir.AluOpType.mult)

            # l scales replicated across partitions (DMA broadcast from DRAM)
            reps_l = prop.tile([P, RC, O], BF16)
            nc.sync.dma_start(
                out=reps_l[:],
                in_=lsF[0:1, None, :, :].broadcast_to([1, P, RC, O]))
            nc.vector.tensor_tensor(out=lt[:], in0=lt[:], in1=reps_l[:],
                                    op=mybir.AluOpType.mult)

            # xr.T = (x @ r_deq.T).T via r_chunk @ x.T, x streamed bf16
            TH = min(512, T)
            NTH = T // TH
            NRH = R // P
            GBLK = min(8, KC)
            pxr = [[psm.tile([P, TH], F32, tag="mm", name=f"pxr_{rh}_{th}")
                    for th in range(NTH)]
                   for rh in range(NRH)]
            for gb in range(KC // GBLK):
                xc = qtp.tile([P, GBLK, T], BF16, tag="xbf")
                nc.sync.dma_start(out=xc[:],
                                  in_=xTb[:, gb * GBLK:(gb + 1) * GBLK, :])
                for gg in range(GBLK):
                    g = gb * GBLK + gg
                    for rh in range(NRH):
                        for th in range(NTH):
                            nc.tensor.matmul(
                                pxr[rh][th][:],
                                lhsT=rt[:, g, rh * P:(rh + 1) * P],
                                rhs=xc[:, gg, th * TH:(th + 1) * TH],
                                start=(g == 0), stop=(g == KC - 1),
                            )
            for rh in range(NRH):
                for th in range(NTH):
                    nc.scalar.copy(xrT[:, rh, th * TH:(th + 1) * TH],
                                   pxr[rh][th][:])

        # ---- main loop over output slabs ----
        with tc.tile_pool(name="repp", bufs=2) as repp:
            for ob in range(NOB):
                qsl = qtp.tile([P, KC, OBW], FP8, tag="q")
                nc.sync.dma_start(out=qsl[:], in_=qT8[ob])
                # replicate scales across partitions: stride-0 DMA from DRAM
                rep = repp.tile([P, KC, OBW], FP8, tag="rep")
                nc.sync.dma_start(
                    out=rep[:],
                    in_=qsS[ob][0:1, None, :, :].broadcast_to([1, P, KC, OBW]))
                # dequant: one DVE multiply, fp8 in place
                nc.vector.tensor_tensor(out=qsl[:], in0=qsl[:], in1=rep[:],
                                        op=mybir.AluOpType.mult)
                # matmuls
                for t in range(TS):
                    ps = psm.tile([P, OBW], F32, tag="mm")
                    for gp in range(NGP):
                        nc.tensor.matmul(
                            ps[:],
                            lhsT=x8[:, 2 * gp:2 * gp + 2, t * P:(t + 1) * P],
                            rhs=qsl[:, 2 * gp:2 * gp + 2, :],
                            start=(gp == 0), stop=False, perf_mode=DR,
                        )
                    for c in range(RC):
                        nc.tensor.matmul(
                            ps[:],
                            lhsT=xrT[:, c, t * P:(t + 1) * P],
                            rhs=lt[:, c, ob * OBW:(ob + 1) * OBW],
                            start=False, stop=(c == RC - 1),
                        )
                    osb = outp.tile([P, OBW], F32)
                    nc.vector.tensor_tensor(
                        out=osb[:], in0=ps[:],
                        in1=bias_bc[:, ob * OBW:(ob + 1) * OBW],
                        op=mybir.AluOpType.add)
                    nc.sync.dma_start(
                        out=out[t * P:(t + 1) * P, ob * OBW:(ob + 1) * OBW],
                        in_=osb[:],
                    )


def build_nc(T=T_SH, O=D_OUT, K=D_IN, R=RANK, obw=512):
    nc = bacc.Bacc("TRN2", target_bir_lowering=False, debug=False)
    KC = K // P
    RC = R // P
    NOB = O // obw
    xT8 = nc.dram_tensor("xT8", [P, KC, T], FP8, kind="ExternalInput").ap()
    xTb = nc.dram_tensor("xTb", [P, KC, T], BF16, kind="ExternalInput").ap()
    qT8 = nc.dram_tensor("qT8", [NOB, P, KC, obw], FP8, kind="ExternalInput").ap()
    qsS = nc.dram_tensor("qsS", [NOB, 1, KC, obw], FP8, kind="ExternalInput").ap()
    rT = nc.dram_tensor("rT", [P, KC, R], BF16, kind="ExternalInput").ap()
    rsF = nc.dram_tensor("rsF", [1, KC, R], BF16, kind="ExternalInput").ap()
    lT = nc.dram_tensor("lT", [P, RC, O], BF16, kind="ExternalInput").ap()
    lsF = nc.dram_tensor("lsF", [1, RC, O], BF16, kind="ExternalInput").ap()
    bias_ = nc.dram_tensor("bias", [1, O], BF16, kind="ExternalInput").ap()
    out = nc.dram_tensor("out", [T, O], F32, kind="ExternalOutput").ap()
    with tile.TileContext(nc) as tc:
        caldera_v4(tc, out, xT8, xTb, qT8, qsS, rT, rsF, lT, lsF, bias_,
                   obw=obw)
    nc.compile()
    return nc


def _chunked_T(a, part=P):
    """[N, K] -> [P, K//P, N] with out[p, g, n] = a[n, g*P+p]."""
    n, k = a.shape
    return np.ascontiguousarray(a.T.reshape(k // part, part, n).transpose(1, 0, 2))


def make_in_maps(x, q_values, q_scales, l_values, l_scales, r_values, r_scales,
                 bias, obw=512):
    bf16 = ml_dtypes.bfloat16
    f8 = ml_dtypes.float8_e4m3
    KC = D_IN // P
    NOB = D_OUT // obw

    xf = np.asarray(x, dtype=np.float32).reshape(N_TOK, D_IN)
    qv = np.asarray(q_values)
    # q.T slab-major fp8: [NOB, P, KC, OBW]
    qT = np.ascontiguousarray(
        qv.T.reshape(KC, P, NOB, obw).transpose(2, 1, 0, 3)).astype(f8)
    # q scales slab-major on partition 0: [NOB, 1, KC, OBW]
    qsS = np.ascontiguousarray(
        np.asarray(q_scales, dtype=np.float32).T  # [KC, O]
        .reshape(KC, NOB, obw).transpose(1, 0, 2)[:, None]).astype(f8)
    rT = _chunked_T(np.asarray(r_values).astype(np.float32)).astype(bf16)
    rsF = np.ascontiguousarray(
        np.asarray(r_scales, dtype=np.float32).T[None]).astype(bf16)
    lT = _chunked_T(np.asarray(l_values).astype(np.float32)).astype(bf16)
    lsF = np.ascontiguousarray(
        np.asarray(l_scales, dtype=np.float32).T[None]).astype(bf16)
    b = np.asarray(bias, dtype=np.float32).reshape(1, D_OUT).astype(bf16)

    in_maps = []
    for i in range(N_CORES):
        xs = xf[i * T_SH:(i + 1) * T_SH]
        xT = _chunked_T(xs)
        in_maps.append({
            "xT8": xT.astype(f8),
            "xTb": xT.astype(bf16),
            "qT8": qT, "qsS": qsS,
            "rT": rT, "rsF": rsF, "lT": lT, "lsF": lsF,
            "bias": b,
        })
    return in_maps


_NC_CACHE = {}


def _get_nc():
    if "nc" not in _NC_CACHE:
        _NC_CACHE["nc"] = build_nc()
    return _NC_CACHE["nc"]


def run(inputs, trace=False, tmpdir=None):
    nc = _get_nc()
    in_maps = make_in_maps(**inputs)
    res = run_bass_kernel_spmd(
        nc, in_maps, list(range(N_CORES)), trace=trace, tmpdir=tmpdir
    )
    shards = [np.asarray(res.results[i]["out"]) for i in range(N_CORES)]
    full = np.concatenate(shards, axis=0).reshape(B, S, D_OUT)
    return full.astype(np.float32), res


def kernel(**inputs) -> np.ndarray:
    out, _ = run(inputs, trace=False)
    return out


# revision 15
# speedup vs baseline: 2.2417x; 1.2074x over previous
"""CalderaLinear Trainium2 kernel, v5 (fp8 DoubleRow q-path, pipelined).

Computes out = x @ dequant(q).T + (x @ dequant(r).T) @ dequant(l).T + bias
with groupwise (group=128) dequantization, distributed over 8 NeuronCores
by sharding tokens (batch*seq) 8 ways and replicating the weights.

Device work: dequant (scale replication via stride-0 broadcast DMA + a
tensor multiply split across the vector and gpsimd engines), the q-path
GEMM in fp8 DoubleRow mode (2 contraction groups per instruction), the
low-rank path in bf16 (quantization noise there rides the coherent mean
components of r/l and is heavily amplified, so it stays 16-bit), bias add
fused into the PSUM drain. Q slabs are software-pipelined 2 deep.

Host work (layout/cast only): shard tokens, transpose weights/activations
to contraction-major layouts, lossless int->fp8/bf16 casts of quantized
values (0..15 exact in e4m3 and bf16), fp32->fp8/bf16 casts of x, scale
transposes/casts, final concat of per-core output shards.
"""

import os
import sys

import numpy as np
import ml_dtypes

for _p in ("/opt/trn_rl_repo",):
    if _p not in sys.path and os.path.isdir(_p):
        sys.path.insert(0, _p)

import concourse.bass as bass
import concourse.mybir as mybir
import concourse.tile as tile
from concourse import bacc
from concourse.bass_utils import run_bass_kernel_spmd

BF16 = mybir.dt.bfloat16
F32 = mybir.dt.float32
FP8 = mybir.dt.float8e4
DR = mybir.MatmulPerfMode.DoubleRow

P = 128  # partitions / quant group size
N_CORES = 8

# Full problem shape (hardcoded per contest contract).
B, S, D_IN, D_OUT, RANK = 4, 2048, 4096, 4096, 256
N_TOK = B * S  # 8192
T_SH = N_TOK // N_CORES  # 1024 tokens per core


def caldera_v5(tc, out, xT8, xTb, qT8, qsS, rT, rsF, lT, lsF, bias_, obw=512):
    """One core's program. DRAM layouts (K=in, O=out, R=rank, T=tokens):
    xT8  [P, K/P, T]        fp8   x.T chunks: xT8[p,g,t] = x[t, g*P+p]
    xTb  [P, K/P, T]        bf16  same values in bf16 (low-rank path input)
    qT8  [NOB, P, K/P, OBW] fp8   q.T slab-major: [ob,p,g,oo] = q[ob*OBW+oo, g*P+p]
    qsS  [NOB, 1, K/P, OBW] fp8   q_scales slab-major: [ob,0,g,oo] = qs[ob*OBW+oo, g]
    rT   [P, K/P, R]        bf16  r.T chunks
    rsF  [1, K/P, R]        bf16  r_scales.T (flat, partition 0)
    lT   [P, R/P, O]        bf16  l.T chunks
    lsF  [1, R/P, O]        bf16  l_scales.T (flat, partition 0)
    bias_ [1, O]            bf16
    out  [T, O]             f32
    """
    nc = tc.nc
    T = xT8.shape[2]
    KC = xT8.shape[1]          # contraction groups (32)
    RC = lT.shape[1]           # rank groups (2)
    R = rT.shape[2]
    O = lT.shape[2]
    OBW = obw
    NOB = O // OBW             # output slabs (8)
    TS = T // P                # token tiles (8)
    NGP = KC // 2              # DoubleRow group pairs (16)
    KH = KC // 2               # half the groups (DMA split)
    CH = max(1, KC // 4)       # dequant chunk (2 on DVE, 2 on gpsimd)

    with tc.tile_pool(name="const", bufs=1) as constp, \
         tc.tile_pool(name="qslp", bufs=3) as qslp, \
         tc.tile_pool(name="repp", bufs=2) as repp, \
         tc.tile_pool(name="xbfp", bufs=2) as xbfp, \
         tc.tile_pool(name="outp", bufs=4) as outp, \
         tc.tile_pool(name="psm", bufs=8, space="PSUM") as psm:

        # ---- resident tensors ----
        x8 = constp.tile([P, KC, T], FP8)
        rt = constp.tile([P, KC, R], BF16)
        lt = constp.tile([P, RC, O], BF16)
        xrT = constp.tile([P, RC, T], BF16)
        bias_bc = constp.tile([P, O], BF16)

        def slab_load(ob):
            """Prefetch + dequantize q slab ob. Returns the fp8 tile."""
            qsl = qslp.tile([P, KC, OBW], FP8, tag="q", name=f"qsl_{ob}")
            rep = repp.tile([P, KC, OBW], FP8, tag="rep", name=f"rep_{ob}")
            for h in range(2):
                nc.sync.dma_start(out=qsl[:, h * KH:(h + 1) * KH, :],
                                  in_=qT8[ob][:, h * KH:(h + 1) * KH, :])
                nc.sync.dma_start(
                    out=rep[:, h * KH:(h + 1) * KH, :],
                    in_=qsS[ob][0:1, None, h * KH:(h + 1) * KH, :]
                    .broadcast_to([1, P, KH, OBW]))
            for ci, eng in ((0, nc.vector), (1, nc.gpsimd),
                            (2, nc.vector), (3, nc.gpsimd)):
                sl = slice(ci * CH, (ci + 1) * CH)
                eng.tensor_tensor(out=qsl[:, sl, :], in0=qsl[:, sl, :],
                                  in1=rep[:, sl, :],
                                  op=mybir.AluOpType.mult)
            return qsl

        # bias broadcast straight from DRAM (stride-0 read)
        nc.sync.dma_start(out=bias_bc[:],
                          in_=bias_[0:1, None, :].broadcast_to([1, P, O]))

        # ---- prologue (scoped pool; space reclaimed for main loop) ----
        with tc.tile_pool(name="pro", bufs=1) as prop:
            # r scales replicated across partitions (DMA broadcast from DRAM)
            reps_r = prop.tile([P, KC, R], BF16, tag="reps")
            nc.sync.dma_start(
                out=reps_r[:],
                in_=rsF[0:1, None, :, :].broadcast_to([1, P, KC, R]))
            nc.sync.dma_start(out=rt[:], in_=rT[:])
            nc.sync.dma_start(out=x8[:], in_=xT8[:])
            nc.sync.dma_start(out=lt[:], in_=lT[:])

            # r dequant in one DVE op
            nc.vector.tensor_tensor(out=rt[:], in0=rt[:], in1=reps_r[:],
                                    op=mybir.AluOpType.mult)

            # l scales replicated across partitions (same pro buffer reused)
            reps_l = prop.tile([P, RC, O], BF16, tag="reps", name="reps_l")
            nc.sync.dma_start(
                out=reps_l[:],
                in_=lsF[0:1, None, :, :].broadcast_to([1, P, RC, O]))
            nc.vector.tensor_tensor(out=lt[:], in0=lt[:], in1=reps_l[:],
                                    op=mybir.AluOpType.mult)

            # prefetch + dequant first two q slabs during the prologue
            slabs = [slab_load(0), slab_load(1)]

            # xr.T = (x @ r_deq.T).T via r_chunk @ x.T, x streamed bf16
            TH = min(512, T)
            NTH = T // TH
            NRH = R // P
            GBLK = min(4, KC)
            pxr = [[psm.tile([P, TH], F32, tag="mm", name=f"pxr_{rh}_{th}")
                    for th in range(NTH)]
                   for rh in range(NRH)]
            for gb in range(KC // GBLK):
                xc = xbfp.tile([P, GBLK, T], BF16, tag="xbf")
                nc.sync.dma_start(out=xc[:],
                                  in_=xTb[:, gb * GBLK:(gb + 1) * GBLK, :])
                for gg in range(GBLK):
                    g = gb * GBLK + gg
                    for rh in range(NRH):
                        for th in range(NTH):
                            nc.tensor.matmul(
                                pxr[rh][th][:],
                                lhsT=rt[:, g, rh * P:(rh + 1) * P],
                                rhs=xc[:, gg, th * TH:(th + 1) * TH],
                                start=(g == 0), stop=(g == KC - 1),
                            )
            for rh in range(NRH):
                for th in range(NTH):
                    nc.scalar.copy(xrT[:, rh, th * TH:(th + 1) * TH],
                                   pxr[rh][th][:])

        # ---- main loop over output slabs (2-deep slab pipeline) ----
        for ob in range(NOB):
            qsl = slabs[ob]
            for t in range(TS):
                ps = psm.tile([P, OBW], F32, tag="mm")
                for gp in range(NGP):
                    nc.tensor.matmul(
                        ps[:],
                        lhsT=x8[:, 2 * gp:2 * gp + 2, t * P:(t + 1) * P],
                        rhs=qsl[:, 2 * gp:2 * gp + 2, :],
                        start=(gp == 0), stop=False, perf_mode=DR,
                    )
                for c in range(RC):
                    nc.tensor.matmul(
                        ps[:],
                        lhsT=xrT[:, c, t * P:(t + 1) * P],
                        rhs=lt[:, c, ob * OBW:(ob + 1) * OBW],
                        start=False, stop=(c == RC - 1),
                    )
                osb = outp.tile([P, OBW], F32)
                nc.vector.tensor_tensor(
                    out=osb[:], in0=ps[:],
                    in1=bias_bc[:, ob * OBW:(ob + 1) * OBW],
                    op=mybir.AluOpType.add)
                nc.sync.dma_start(
                    out=out[t * P:(t + 1) * P, ob * OBW:(ob + 1) * OBW],
                    in_=osb[:],
                )
                if t == 0 and ob + 2 < NOB:
                    slabs.append(slab_load(ob + 2))


def build_nc(T=T_SH, O=D_OUT, K=D_IN, R=RANK, obw=512):
    nc = bacc.Bacc("TRN2", target_bir_lowering=False, debug=False)
    KC = K // P
    RC = R // P
    NOB = O // obw
    xT8 = nc.dram_tensor("xT8", [P, KC, T], FP8, kind="ExternalInput").ap()
    xTb = nc.dram_tensor("xTb", [P, KC, T], BF16, kind="ExternalInput").ap()
    qT8 = nc.dram_tensor("qT8", [NOB, P, KC, obw], FP8, kind="ExternalInput").ap()
    qsS = nc.dram_tensor("qsS", [NOB, 1, KC, obw], FP8, kind="ExternalInput").ap()
    rT = nc.dram_tensor("rT", [P, KC, R], BF16, kind="ExternalInput").ap()
    rsF = nc.dram_tensor("rsF", [1, KC, R], BF16, kind="ExternalInput").ap()
    lT = nc.dram_tensor("lT", [P, RC, O], BF16, kind="ExternalInput").ap()
    lsF = nc.dram_tensor("lsF", [1, RC, O], BF16, kind="ExternalInput").ap()
    bias_ = nc.dram_tensor("bias", [1, O], BF16, kind="ExternalInput").ap()
    out = nc.dram_tensor("out", [T, O], F32, kind="ExternalOutput").ap()
    with tile.TileContext(nc) as tc:
        caldera_v5(tc, out, xT8, xTb, qT8, qsS, rT, rsF, lT, lsF, bias_,
                   obw=obw)
    nc.compile()
    return nc


def _chunked_T(a, part=P):
    """[N, K] -> [P, K//P, N] with out[p, g, n] = a[n, g*P+p]."""
    n, k = a.shape
    return np.ascontiguousarray(a.T.reshape(k // part, part, n).transpose(1, 0, 2))


def make_in_maps(x, q_values, q_scales, l_values, l_scales, r_values, r_scales,
                 bias, obw=512):
    bf16 = ml_dtypes.bfloat16
    f8 = ml_dtypes.float8_e4m3
    KC = D_IN // P
    NOB = D_OUT // obw

    xf = np.asarray(x, dtype=np.float32).reshape(N_TOK, D_IN)
    qv = np.asarray(q_values)
    # q.T slab-major fp8: [NOB, P, KC, OBW]
    qT = np.ascontiguousarray(
        qv.T.reshape(KC, P, NOB, obw).transpose(2, 1, 0, 3)).astype(f8)
    # q scales slab-major on partition 0: [NOB, 1, KC, OBW]
    qsS = np.ascontiguousarray(
        np.asarray(q_scales, dtype=np.float32).T  # [KC, O]
        .reshape(KC, NOB, obw).transpose(1, 0, 2)[:, None]).astype(f8)
    rT = _chunked_T(np.asarray(r_values).astype(np.float32)).astype(bf16)
    rsF = np.ascontiguousarray(
        np.asarray(r_scales, dtype=np.float32).T[None]).astype(bf16)
    lT = _chunked_T(np.asarray(l_values).astype(np.float32)).astype(bf16)
    lsF = np.ascontiguousarray(
        np.asarray(l_scales, dtype=np.float32).T[None]).astype(bf16)
    b = np.asarray(bias, dtype=np.float32).reshape(1, D_OUT).astype(bf16)

    in_maps = []
    for i in range(N_CORES):
        xs = xf[i * T_SH:(i + 1) * T_SH]
        xT = _chunked_T(xs)
        in_maps.append({
            "xT8": xT.astype(f8),
            "xTb": xT.astype(bf16),
            "qT8": qT, "qsS": qsS,
            "rT": rT, "rsF": rsF, "lT": lT, "lsF": lsF,
            "bias": b,
        })
    return in_maps


_NC_CACHE = {}


def _get_nc():
    if "nc" not in _NC_CACHE:
        _NC_CACHE["nc"] = build_nc()
    return _NC_CACHE["nc"]


def run(inputs, trace=False, tmpdir=None):
    nc = _get_nc()
    in_maps = make_in_maps(**inputs)
    res = run_bass_kernel_spmd(
        nc, in_maps, list(range(N_CORES)), trace=trace, tmpdir=tmpdir
    )
    shards = [np.asarray(res.results[i]["out"]) for i in range(N_CORES)]
    full = np.concatenate(shards, axis=0).reshape(B, S, D_OUT)
    return full.astype(np.float32), res


def kernel(**inputs) -> np.ndarray:
    out, _ = run(inputs, trace=False)
    return out


# revision 16
# speedup vs baseline: 2.2548x; 1.0058x over previous
"""CalderaLinear Trainium2 kernel, v5 (fp8 DoubleRow q-path, pipelined).

Computes out = x @ dequant(q).T + (x @ dequant(r).T) @ dequant(l).T + bias
with groupwise (group=128) dequantization, distributed over 8 NeuronCores
by sharding tokens (batch*seq) 8 ways and replicating the weights.

Device work: dequant (scale replication via stride-0 broadcast DMA + a
tensor multiply split across the vector and gpsimd engines), the q-path
GEMM in fp8 DoubleRow mode (2 contraction groups per instruction), the
low-rank path in bf16 (quantization noise there rides the coherent mean
components of r/l and is heavily amplified, so it stays 16-bit), bias add
fused into the PSUM drain. Q slabs are software-pipelined 2 deep.

Host work (layout/cast only): shard tokens, transpose weights/activations
to contraction-major layouts, lossless int->fp8/bf16 casts of quantized
values (0..15 exact in e4m3 and bf16), fp32->fp8/bf16 casts of x, scale
transposes/casts, final concat of per-core output shards.
"""

import os
import sys

import numpy as np
import ml_dtypes

for _p in ("/opt/trn_rl_repo",):
    if _p not in sys.path and os.path.isdir(_p):
        sys.path.insert(0, _p)

import concourse.bass as bass
import concourse.mybir as mybir
import concourse.tile as tile
from concourse import bacc
from concourse.bass_utils import run_bass_kernel_spmd

BF16 = mybir.dt.bfloat16
F32 = mybir.dt.float32
FP8 = mybir.dt.float8e4
DR = mybir.MatmulPerfMode.DoubleRow

P = 128  # partitions / quant group size
N_CORES = 8

# Full problem shape (hardcoded per contest contract).
B, S, D_IN, D_OUT, RANK = 4, 2048, 4096, 4096, 256
N_TOK = B * S  # 8192
T_SH = N_TOK // N_CORES  # 1024 tokens per core


def caldera_v5(tc, out, xT8, xTb, qT8, qsS, rT, rsF, lT, lsF, bias_, obw=512):
    """One core's program. DRAM layouts (K=in, O=out, R=rank, T=tokens):
    xT8  [P, K/P, T]        fp8   x.T chunks: xT8[p,g,t] = x[t, g*P+p]
    xTb  [P, K/P, T]        bf16  same values in bf16 (low-rank path input)
    qT8  [NOB, P, K/P, OBW] fp8   q.T slab-major: [ob,p,g,oo] = q[ob*OBW+oo, g*P+p]
    qsS  [NOB, 1, K/P, OBW] fp8   q_scales slab-major: [ob,0,g,oo] = qs[ob*OBW+oo, g]
    rT   [P, K/P, R]        bf16  r.T chunks
    rsF  [1, K/P, R]        bf16  r_scales.T (flat, partition 0)
    lT   [P, R/P, O]        bf16  l.T chunks
    lsF  [1, R/P, O]        bf16  l_scales.T (flat, partition 0)
    bias_ [1, O]            bf16
    out  [T, O]             f32
    """
    nc = tc.nc
    T = xT8.shape[2]
    KC = xT8.shape[1]          # contraction groups (32)
    RC = lT.shape[1]           # rank groups (2)
    R = rT.shape[2]
    O = lT.shape[2]
    OBW = obw
    NOB = O // OBW             # output slabs (8)
    TS = T // P                # token tiles (8)
    NGP = KC // 2              # DoubleRow group pairs (16)
    KH = KC // 2               # half the groups (DMA split)
    CH = max(1, KC // 4)       # dequant chunk (2 on DVE, 2 on gpsimd)

    with tc.tile_pool(name="const", bufs=1) as constp, \
         tc.tile_pool(name="qslp", bufs=3) as qslp, \
         tc.tile_pool(name="repp", bufs=2) as repp, \
         tc.tile_pool(name="xbfp", bufs=2) as xbfp, \
         tc.tile_pool(name="outp", bufs=4) as outp, \
         tc.tile_pool(name="psm", bufs=8, space="PSUM") as psm:

        # ---- resident tensors ----
        x8 = constp.tile([P, KC, T], FP8)
        rt = constp.tile([P, KC, R], BF16)
        lt = constp.tile([P, RC, O], BF16)
        xrT = constp.tile([P, RC, T], BF16)
        bias_bc = constp.tile([P, O], BF16)

        def slab_load(ob):
            """Prefetch + dequantize q slab ob. Returns the fp8 tile."""
            qsl = qslp.tile([P, KC, OBW], FP8, tag="q", name=f"qsl_{ob}")
            rep = repp.tile([P, KC, OBW], FP8, tag="rep", name=f"rep_{ob}")
            for h in range(2):
                nc.sync.dma_start(out=qsl[:, h * KH:(h + 1) * KH, :],
                                  in_=qT8[ob][:, h * KH:(h + 1) * KH, :])
                nc.sync.dma_start(
                    out=rep[:, h * KH:(h + 1) * KH, :],
                    in_=qsS[ob][0:1, None, h * KH:(h + 1) * KH, :]
                    .broadcast_to([1, P, KH, OBW]))
            for ci, eng in ((0, nc.vector), (1, nc.gpsimd),
                            (2, nc.vector), (3, nc.gpsimd)):
                sl = slice(ci * CH, (ci + 1) * CH)
                eng.tensor_tensor(out=qsl[:, sl, :], in0=qsl[:, sl, :],
                                  in1=rep[:, sl, :],
                                  op=mybir.AluOpType.mult)
            return qsl

        # ---- prologue (scoped pool; space reclaimed for main loop) ----
        # DMA emission order tracks need-order: the xr phase needs
        # rt + r scales + the first x chunks; slab 0 / x8 are needed at
        # main-loop start; lt / bias / slab 1 only a few us into it.
        with tc.tile_pool(name="pro", bufs=1) as prop:
            # r scales replicated across partitions (DMA broadcast from DRAM)
            reps_r = prop.tile([P, KC, R], BF16, tag="reps")
            nc.sync.dma_start(
                out=reps_r[:],
                in_=rsF[0:1, None, :, :].broadcast_to([1, P, KC, R]))
            nc.sync.dma_start(out=rt[:], in_=rT[:])

            # first two x chunks for the xr phase
            TH = min(512, T)
            NTH = T // TH
            NRH = R // P
            GBLK = min(4, KC)
            NGB = KC // GBLK
            xcs = {}
            for gb in range(min(2, NGB)):
                xcs[gb] = xbfp.tile([P, GBLK, T], BF16, tag="xbf",
                                    name=f"xc_{gb}")
                nc.sync.dma_start(out=xcs[gb][:],
                                  in_=xTb[:, gb * GBLK:(gb + 1) * GBLK, :])

            # r dequant in one DVE op
            nc.vector.tensor_tensor(out=rt[:], in0=rt[:], in1=reps_r[:],
                                    op=mybir.AluOpType.mult)

            # slab 0 + x8 (needed at main-loop start)
            slabs = [slab_load(0)]
            nc.sync.dma_start(out=x8[:], in_=xT8[:])

            # l / bias / slab 1 (needed a few us into the main loop)
            nc.sync.dma_start(out=lt[:], in_=lT[:])
            reps_l = prop.tile([P, RC, O], BF16, tag="reps", name="reps_l")
            nc.sync.dma_start(
                out=reps_l[:],
                in_=lsF[0:1, None, :, :].broadcast_to([1, P, RC, O]))
            # l dequant split across vector/gpsimd so neither queue blocks
            for cg, eng in ((0, nc.vector), (1, nc.gpsimd)):
                eng.tensor_tensor(out=lt[:, cg, :], in0=lt[:, cg, :],
                                  in1=reps_l[:, cg, :],
                                  op=mybir.AluOpType.mult)
            nc.sync.dma_start(out=bias_bc[:],
                              in_=bias_[0:1, None, :].broadcast_to([1, P, O]))
            slabs.append(slab_load(1))

            # xr.T = (x @ r_deq.T).T via r_chunk @ x.T, x streamed bf16
            pxr = [[psm.tile([P, TH], F32, tag="mm", name=f"pxr_{rh}_{th}")
                    for th in range(NTH)]
                   for rh in range(NRH)]
            for gb in range(NGB):
                if gb not in xcs:
                    xcs[gb] = xbfp.tile([P, GBLK, T], BF16, tag="xbf",
                                        name=f"xc_{gb}")
                    nc.sync.dma_start(
                        out=xcs[gb][:],
                        in_=xTb[:, gb * GBLK:(gb + 1) * GBLK, :])
                xc = xcs[gb]
                for gg in range(GBLK):
                    g = gb * GBLK + gg
                    for rh in range(NRH):
                        for th in range(NTH):
                            nc.tensor.matmul(
                                pxr[rh][th][:],
                                lhsT=rt[:, g, rh * P:(rh + 1) * P],
                                rhs=xc[:, gg, th * TH:(th + 1) * TH],
                                start=(g == 0), stop=(g == KC - 1),
                            )
            for rh in range(NRH):
                for th in range(NTH):
                    nc.scalar.copy(xrT[:, rh, th * TH:(th + 1) * TH],
                                   pxr[rh][th][:])

        # ---- main loop over output slabs (2-deep slab pipeline) ----
        for ob in range(NOB):
            qsl = slabs[ob]
            for t in range(TS):
                ps = psm.tile([P, OBW], F32, tag="mm")
                for gp in range(NGP):
                    nc.tensor.matmul(
                        ps[:],
                        lhsT=x8[:, 2 * gp:2 * gp + 2, t * P:(t + 1) * P],
                        rhs=qsl[:, 2 * gp:2 * gp + 2, :],
                        start=(gp == 0), stop=False, perf_mode=DR,
                    )
                for c in range(RC):
                    nc.tensor.matmul(
                        ps[:],
                        lhsT=xrT[:, c, t * P:(t + 1) * P],
                        rhs=lt[:, c, ob * OBW:(ob + 1) * OBW],
                        start=False, stop=(c == RC - 1),
                    )
                osb = outp.tile([P, OBW], F32)
                nc.vector.tensor_tensor(
                    out=osb[:], in0=ps[:],
                    in1=bias_bc[:, ob * OBW:(ob + 1) * OBW],
                    op=mybir.AluOpType.add)
                nc.sync.dma_start(
                    out=out[t * P:(t + 1) * P, ob * OBW:(ob + 1) * OBW],
                    in_=osb[:],
                )
                if t == 0 and ob + 2 < NOB:
                    slabs.append(slab_load(ob + 2))


def build_nc(T=T_SH, O=D_OUT, K=D_IN, R=RANK, obw=512):
    nc = bacc.Bacc("TRN2", target_bir_lowering=False, debug=False)
    KC = K // P
    RC = R // P
    NOB = O // obw
    xT8 = nc.dram_tensor("xT8", [P, KC, T], FP8, kind="ExternalInput").ap()
    xTb = nc.dram_tensor("xTb", [P, KC, T], BF16, kind="ExternalInput").ap()
    qT8 = nc.dram_tensor("qT8", [NOB, P, KC, obw], FP8, kind="ExternalInput").ap()
    qsS = nc.dram_tensor("qsS", [NOB, 1, KC, obw], FP8, kind="ExternalInput").ap()
    rT = nc.dram_tensor("rT", [P, KC, R], BF16, kind="ExternalInput").ap()
    rsF = nc.dram_tensor("rsF", [1, KC, R], BF16, kind="ExternalInput").ap()
    lT = nc.dram_tensor("lT", [P, RC, O], BF16, kind="ExternalInput").ap()
    lsF = nc.dram_tensor("lsF", [1, RC, O], BF16, kind="ExternalInput").ap()
    bias_ = nc.dram_tensor("bias", [1, O], BF16, kind="ExternalInput").ap()
    out = nc.dram_tensor("out", [T, O], F32, kind="ExternalOutput").ap()
    with tile.TileContext(nc) as tc:
        caldera_v5(tc, out, xT8, xTb, qT8, qsS, rT, rsF, lT, lsF, bias_,
                   obw=obw)
    nc.compile()
    return nc


def _chunked_T(a, part=P):
    """[N, K] -> [P, K//P, N] with out[p, g, n] = a[n, g*P+p]."""
    n, k = a.shape
    return np.ascontiguousarray(a.T.reshape(k // part, part, n).transpose(1, 0, 2))


def make_in_maps(x, q_values, q_scales, l_values, l_scales, r_values, r_scales,
                 bias, obw=512):
    bf16 = ml_dtypes.bfloat16
    f8 = ml_dtypes.float8_e4m3
    KC = D_IN // P
    NOB = D_OUT // obw

    xf = np.asarray(x, dtype=np.float32).reshape(N_TOK, D_IN)
    qv = np.asarray(q_values)
    # q.T slab-major fp8: [NOB, P, KC, OBW]
    qT = np.ascontiguousarray(
        qv.T.reshape(KC, P, NOB, obw).transpose(2, 1, 0, 3)).astype(f8)
    # q scales slab-major on partition 0: [NOB, 1, KC, OBW]
    qsS = np.ascontiguousarray(
        np.asarray(q_scales, dtype=np.float32).T  # [KC, O]
        .reshape(KC, NOB, obw).transpose(1, 0, 2)[:, None]).astype(f8)
    rT = _chunked_T(np.asarray(r_values).astype(np.float32)).astype(bf16)
    rsF = np.ascontiguousarray(
        np.asarray(r_scales, dtype=np.float32).T[None]).astype(bf16)
    lT = _chunked_T(np.asarray(l_values).astype(np.float32)).astype(bf16)
    lsF = np.ascontiguousarray(
        np.asarray(l_scales, dtype=np.float32).T[None]).astype(bf16)
    b = np.asarray(bias, dtype=np.float32).reshape(1, D_OUT).astype(bf16)

    in_maps = []
    for i in range(N_CORES):
        xs = xf[i * T_SH:(i + 1) * T_SH]
        xT = _chunked_T(xs)
        in_maps.append({
            "xT8": xT.astype(f8),
            "xTb": xT.astype(bf16),
            "qT8": qT, "qsS": qsS,
            "rT": rT, "rsF": rsF, "lT": lT, "lsF": lsF,
            "bias": b,
        })
    return in_maps


_NC_CACHE = {}


def _get_nc():
    if "nc" not in _NC_CACHE:
        _NC_CACHE["nc"] = build_nc()
    return _NC_CACHE["nc"]


def run(inputs, trace=False, tmpdir=None):
    nc = _get_nc()
    in_maps = make_in_maps(**inputs)
    res = run_bass_kernel_spmd(
        nc, in_maps, list(range(N_CORES)), trace=trace, tmpdir=tmpdir
    )
    shards = [np.asarray(res.results[i]["out"]) for i in range(N_CORES)]
    full = np.concatenate(shards, axis=0).reshape(B, S, D_OUT)
    return full.astype(np.float32), res


def kernel(**inputs) -> np.ndarray:
    out, _ = run(inputs, trace=False)
    return out


# revision 17
# speedup vs baseline: 2.3576x; 1.0456x over previous
"""CalderaLinear Trainium2 kernel, v5 (fp8 DoubleRow q-path, pipelined).

Computes out = x @ dequant(q).T + (x @ dequant(r).T) @ dequant(l).T + bias
with groupwise (group=128) dequantization, distributed over 8 NeuronCores
by sharding tokens (batch*seq) 8 ways and replicating the weights.

Device work: dequant (scale replication via stride-0 broadcast DMA + a
tensor multiply split across the vector and gpsimd engines), the q-path
GEMM in fp8 DoubleRow mode (2 contraction groups per instruction), the
low-rank path in bf16 (quantization noise there rides the coherent mean
components of r/l and is heavily amplified, so it stays 16-bit), bias add
fused into the PSUM drain. Q slabs are software-pipelined 2 deep.

Host work (layout/cast only): shard tokens, transpose weights/activations
to contraction-major layouts, lossless int->fp8/bf16 casts of quantized
values (0..15 exact in e4m3 and bf16), fp32->fp8/bf16 casts of x, scale
transposes/casts, final concat of per-core output shards.
"""

import os
import sys

import numpy as np
import ml_dtypes

for _p in ("/opt/trn_rl_repo",):
    if _p not in sys.path and os.path.isdir(_p):
        sys.path.insert(0, _p)

import concourse.bass as bass
import concourse.mybir as mybir
import concourse.tile as tile
from concourse import bacc
from concourse.bass_utils import run_bass_kernel_spmd

BF16 = mybir.dt.bfloat16
F32 = mybir.dt.float32
FP8 = mybir.dt.float8e4
DR = mybir.MatmulPerfMode.DoubleRow

P = 128  # partitions / quant group size
N_CORES = 8

# Full problem shape (hardcoded per contest contract).
B, S, D_IN, D_OUT, RANK = 4, 2048, 4096, 4096, 256
N_TOK = B * S  # 8192
T_SH = N_TOK // N_CORES  # 1024 tokens per core


def caldera_v5(tc, out, xTb, qT8, qsS, rT, rsF, lT, lsF, bias_, obw=512):
    """One core's program. DRAM layouts (K=in, O=out, R=rank, T=tokens):
    xTb  [P, K/P, T]        bf16  x.T chunks: xTb[p,g,t] = x[t, g*P+p]
                                  (fp8 copy for the q path is cast on-device)
    qT8  [NOB, P, K/P, OBW] fp8   q.T slab-major: [ob,p,g,oo] = q[ob*OBW+oo, g*P+p]
    qsS  [NOB, 1, K/P, OBW] fp8   q_scales slab-major: [ob,0,g,oo] = qs[ob*OBW+oo, g]
    rT   [P, K/P, R]        bf16  r.T chunks
    rsF  [1, K/P, R]        bf16  r_scales.T (flat, partition 0)
    lT   [P, R/P, O]        bf16  l.T chunks
    lsF  [1, R/P, O]        bf16  l_scales.T (flat, partition 0)
    bias_ [1, O]            bf16
    out  [T, O]             f32
    """
    nc = tc.nc
    T = xTb.shape[2]
    KC = xTb.shape[1]          # contraction groups (32)
    RC = lT.shape[1]           # rank groups (2)
    R = rT.shape[2]
    O = lT.shape[2]
    OBW = obw
    NOB = O // OBW             # output slabs (8)
    TS = T // P                # token tiles (8)
    NGP = KC // 2              # DoubleRow group pairs (16)
    KH = KC // 2               # half the groups (DMA split)
    CH = max(1, KC // 4)       # dequant chunk (2 on DVE, 2 on gpsimd)

    with tc.tile_pool(name="const", bufs=1) as constp, \
         tc.tile_pool(name="qslp", bufs=3) as qslp, \
         tc.tile_pool(name="repp", bufs=2) as repp, \
         tc.tile_pool(name="xbfp", bufs=2) as xbfp, \
         tc.tile_pool(name="outp", bufs=4) as outp, \
         tc.tile_pool(name="psm", bufs=8, space="PSUM") as psm:

        # ---- resident tensors ----
        x8 = constp.tile([P, KC, T], FP8)
        rt = constp.tile([P, KC, R], BF16)
        lt = constp.tile([P, RC, O], BF16)
        xrT = constp.tile([P, RC, T], BF16)
        bias_bc = constp.tile([P, O], BF16)

        def slab_load(ob):
            """Prefetch + dequantize q slab ob. Returns the fp8 tile."""
            qsl = qslp.tile([P, KC, OBW], FP8, tag="q", name=f"qsl_{ob}")
            rep = repp.tile([P, KC, OBW], FP8, tag="rep", name=f"rep_{ob}")
            for h in range(2):
                nc.sync.dma_start(out=qsl[:, h * KH:(h + 1) * KH, :],
                                  in_=qT8[ob][:, h * KH:(h + 1) * KH, :])
                nc.sync.dma_start(
                    out=rep[:, h * KH:(h + 1) * KH, :],
                    in_=qsS[ob][0:1, None, h * KH:(h + 1) * KH, :]
                    .broadcast_to([1, P, KH, OBW]))
            for ci, eng in ((0, nc.vector), (1, nc.gpsimd),
                            (2, nc.vector), (3, nc.gpsimd)):
                sl = slice(ci * CH, (ci + 1) * CH)
                eng.tensor_tensor(out=qsl[:, sl, :], in0=qsl[:, sl, :],
                                  in1=rep[:, sl, :],
                                  op=mybir.AluOpType.mult)
            return qsl

        # ---- prologue (scoped pool; space reclaimed for main loop) ----
        # The prologue is HBM-bound: only rt/xTb/slab0/lt reads matter.
        # x8 (fp8 x for the q path) is cast on-device from the bf16 x
        # chunks by the otherwise-idle scalar engine, saving 4.2MB of DMA.
        with tc.tile_pool(name="pro", bufs=1) as prop:
            # r scales replicated across partitions (DMA broadcast from DRAM)
            reps_r = prop.tile([P, KC, R], BF16, tag="reps")
            nc.sync.dma_start(
                out=reps_r[:],
                in_=rsF[0:1, None, :, :].broadcast_to([1, P, KC, R]))
            nc.sync.dma_start(out=rt[:], in_=rT[:])

            # first two x chunks for the xr phase
            TH = min(512, T)
            NTH = T // TH
            NRH = R // P
            GBLK = min(4, KC)
            NGB = KC // GBLK

            def xc_load(gb):
                xc = xbfp.tile([P, GBLK, T], BF16, tag="xbf", name=f"xc_{gb}")
                nc.sync.dma_start(out=xc[:],
                                  in_=xTb[:, gb * GBLK:(gb + 1) * GBLK, :])
                # fp8 copy for the main-loop q path (scalar engine cast)
                nc.scalar.copy(x8[:, gb * GBLK:(gb + 1) * GBLK, :], xc[:])
                return xc

            xcs = {gb: xc_load(gb) for gb in range(min(2, NGB))}

            # r dequant in one DVE op
            nc.vector.tensor_tensor(out=rt[:], in0=rt[:], in1=reps_r[:],
                                    op=mybir.AluOpType.mult)

            # slab 0 (needed at main-loop start), l (needed a few us in)
            slabs = [slab_load(0)]
            nc.sync.dma_start(out=lt[:], in_=lT[:])
            reps_l = prop.tile([P, RC, O], BF16, tag="reps", name="reps_l")
            nc.sync.dma_start(
                out=reps_l[:],
                in_=lsF[0:1, None, :, :].broadcast_to([1, P, RC, O]))
            # l dequant split across vector/gpsimd so neither queue blocks
            for cg, eng in ((0, nc.vector), (1, nc.gpsimd)):
                eng.tensor_tensor(out=lt[:, cg, :], in0=lt[:, cg, :],
                                  in1=reps_l[:, cg, :],
                                  op=mybir.AluOpType.mult)
            nc.sync.dma_start(out=bias_bc[:],
                              in_=bias_[0:1, None, :].broadcast_to([1, P, O]))
            slabs.append(slab_load(1))

            # xr.T = (x @ r_deq.T).T via r_chunk @ x.T, x streamed bf16
            pxr = [[psm.tile([P, TH], F32, tag="mm", name=f"pxr_{rh}_{th}")
                    for th in range(NTH)]
                   for rh in range(NRH)]
            for gb in range(NGB):
                if gb not in xcs:
                    xcs[gb] = xc_load(gb)
                xc = xcs[gb]
                for gg in range(GBLK):
                    g = gb * GBLK + gg
                    for rh in range(NRH):
                        for th in range(NTH):
                            nc.tensor.matmul(
                                pxr[rh][th][:],
                                lhsT=rt[:, g, rh * P:(rh + 1) * P],
                                rhs=xc[:, gg, th * TH:(th + 1) * TH],
                                start=(g == 0), stop=(g == KC - 1),
                            )
            for rh in range(NRH):
                for th in range(NTH):
                    nc.scalar.copy(xrT[:, rh, th * TH:(th + 1) * TH],
                                   pxr[rh][th][:])

        # ---- main loop over output slabs (2-deep slab pipeline) ----
        for ob in range(NOB):
            qsl = slabs[ob]
            for t in range(TS):
                ps = psm.tile([P, OBW], F32, tag="mm")
                for gp in range(NGP):
                    nc.tensor.matmul(
                        ps[:],
                        lhsT=x8[:, 2 * gp:2 * gp + 2, t * P:(t + 1) * P],
                        rhs=qsl[:, 2 * gp:2 * gp + 2, :],
                        start=(gp == 0), stop=False, perf_mode=DR,
                    )
                for c in range(RC):
                    nc.tensor.matmul(
                        ps[:],
                        lhsT=xrT[:, c, t * P:(t + 1) * P],
                        rhs=lt[:, c, ob * OBW:(ob + 1) * OBW],
                        start=False, stop=(c == RC - 1),
                    )
                osb = outp.tile([P, OBW], F32)
                nc.vector.tensor_tensor(
                    out=osb[:], in0=ps[:],
                    in1=bias_bc[:, ob * OBW:(ob + 1) * OBW],
                    op=mybir.AluOpType.add)
                nc.sync.dma_start(
                    out=out[t * P:(t + 1) * P, ob * OBW:(ob + 1) * OBW],
                    in_=osb[:],
                )
                if t == 0 and ob + 2 < NOB:
                    slabs.append(slab_load(ob + 2))


def build_nc(T=T_SH, O=D_OUT, K=D_IN, R=RANK, obw=512):
    nc = bacc.Bacc("TRN2", target_bir_lowering=False, debug=False)
    KC = K // P
    RC = R // P
    NOB = O // obw
    xTb = nc.dram_tensor("xTb", [P, KC, T], BF16, kind="ExternalInput").ap()
    qT8 = nc.dram_tensor("qT8", [NOB, P, KC, obw], FP8, kind="ExternalInput").ap()
    qsS = nc.dram_tensor("qsS", [NOB, 1, KC, obw], FP8, kind="ExternalInput").ap()
    rT = nc.dram_tensor("rT", [P, KC, R], BF16, kind="ExternalInput").ap()
    rsF = nc.dram_tensor("rsF", [1, KC, R], BF16, kind="ExternalInput").ap()
    lT = nc.dram_tensor("lT", [P, RC, O], BF16, kind="ExternalInput").ap()
    lsF = nc.dram_tensor("lsF", [1, RC, O], BF16, kind="ExternalInput").ap()
    bias_ = nc.dram_tensor("bias", [1, O], BF16, kind="ExternalInput").ap()
    out = nc.dram_tensor("out", [T, O], F32, kind="ExternalOutput").ap()
    with tile.TileContext(nc) as tc:
        caldera_v5(tc, out, xTb, qT8, qsS, rT, rsF, lT, lsF, bias_,
                   obw=obw)
    nc.compile()
    return nc


def _chunked_T(a, part=P):
    """[N, K] -> [P, K//P, N] with out[p, g, n] = a[n, g*P+p]."""
    n, k = a.shape
    return np.ascontiguousarray(a.T.reshape(k // part, part, n).transpose(1, 0, 2))


def make_in_maps(x, q_values, q_scales, l_values, l_scales, r_values, r_scales,
                 bias, obw=512):
    bf16 = ml_dtypes.bfloat16
    f8 = ml_dtypes.float8_e4m3
    KC = D_IN // P
    NOB = D_OUT // obw

    xf = np.asarray(x, dtype=np.float32).reshape(N_TOK, D_IN)
    qv = np.asarray(q_values)
    # q.T slab-major fp8: [NOB, P, KC, OBW]
    qT = np.ascontiguousarray(
        qv.T.reshape(KC, P, NOB, obw).transpose(2, 1, 0, 3)).astype(f8)
    # q scales slab-major on partition 0: [NOB, 1, KC, OBW]
    qsS = np.ascontiguousarray(
        np.asarray(q_scales, dtype=np.float32).T  # [KC, O]
        .reshape(KC, NOB, obw).transpose(1, 0, 2)[:, None]).astype(f8)
    rT = _chunked_T(np.asarray(r_values).astype(np.float32)).astype(bf16)
    rsF = np.ascontiguousarray(
        np.asarray(r_scales, dtype=np.float32).T[None]).astype(bf16)
    lT = _chunked_T(np.asarray(l_values).astype(np.float32)).astype(bf16)
    lsF = np.ascontiguousarray(
        np.asarray(l_scales, dtype=np.float32).T[None]).astype(bf16)
    b = np.asarray(bias, dtype=np.float32).reshape(1, D_OUT).astype(bf16)

    in_maps = []
    for i in range(N_CORES):
        xs = xf[i * T_SH:(i + 1) * T_SH]
        in_maps.append({
            "xTb": _chunked_T(xs).astype(bf16),
            "qT8": qT, "qsS": qsS,
            "rT": rT, "rsF": rsF, "lT": lT, "lsF": lsF,
            "bias": b,
        })
    return in_maps


_NC_CACHE = {}


def _get_nc():
    if "nc" not in _NC_CACHE:
        _NC_CACHE["nc"] = build_nc()
    return _NC_CACHE["nc"]


def run(inputs, trace=False, tmpdir=None):
    nc = _get_nc()
    in_maps = make_in_maps(**inputs)
    res = run_bass_kernel_spmd(
        nc, in_maps, list(range(N_CORES)), trace=trace, tmpdir=tmpdir
    )
    shards = [np.asarray(res.results[i]["out"]) for i in range(N_CORES)]
    full = np.concatenate(shards, axis=0).reshape(B, S, D_OUT)
    return full.astype(np.float32), res


def kernel(**inputs) -> np.ndarray:
    out, _ = run(inputs, trace=False)
    return out
